# revision 16
# baseline (speedup 1.0000x reference)
"""CalScaleOPTAttention on 8 TRN2 NeuronCores — v3.0.

Sharding: heads across cores (2 heads / core, 256 channels each).

v3 changes vs v2.3 (634us):
- Pass 1 interleaves heads per row-tile; both heads' acc accumulate into
  ONE PSUM row set -> single 8KB AllReduce at pass-1 end (plus a tiny
  warmup AR to absorb core skew before it).
- sv scale path uses [16,CH] contiguous loads + gpsimd partition reduce
  + partition_broadcast (the old svh gather was 4k x 4B descriptor DMAs
  that stalled the acc AllReduce by ~27us).
- k4/k8 int tiles precomputed on vector slack during pass-1; post-topk
  work is just flag broadcast + copy_predicated + one scale mult.
- wo prefetched during pass 1.
- Tail: ctx is AllGathered RAW in bf16 per head (head-0 AG hides under
  pass-2 head-1), cmax AR runs between the AGs, quantization happens
  after the gather on every core, o-proj consumes gathered tiles
  uniformly (no own-tile special case), f1 epilogue runs per chunk.
"""

import numpy as np
import ml_dtypes

import concourse.bass as bass
import concourse.mybir as mybir
import concourse.tile as tile
from concourse import bacc
from concourse import bass_isa
from concourse.bass_utils import run_bass_kernel_spmd

F32 = mybir.dt.float32
BF16 = mybir.dt.bfloat16
I32 = mybir.dt.int32
AX = mybir.AxisListType
OP = mybir.AluOpType
ACTF = mybir.ActivationFunctionType
RED = bass_isa.ReduceOp

NCORES = 8
T = 2048
E = 2048
H = 16
D = 128                   # head dim
HL = H // NCORES          # heads per core = 2
CH = HL * D               # channels per core = 256
NT = T // 128             # 16 row tiles
NE = E // 128             # 16 contraction tiles
Q8 = 127.0
Q4 = 7.0
EPS = 1e-5
NEG = -1e9
RND_C = 12582912.0        # 1.5 * 2**23 round-to-int trick constant
SCALING = float(D) ** -0.5
K_TOP = T // 40           # 51


def _cdiv(a, b):
    return (a + b - 1) // b


def build(mask_mode: str):
    nc = bacc.Bacc("TRN2", target_bir_lowering=False, debug=False,
                   num_devices=NCORES)

    def dt_in(n, s, d):
        return nc.dram_tensor(n, s, d, kind="ExternalInput").ap()

    g = {"mode": mask_mode}
    g["xq_d"] = dt_in("xq", [E, T], BF16)
    g["sx_d"] = dt_in("sx", [T], F32)
    g["sxpp_d"] = dt_in("sxpp", [128, NT], F32)
    for w in ("wq", "wk", "wv", "wo"):
        g[w + "_d"] = dt_in(w, [E, CH], BF16)
    for v in ("swq", "swk", "swo", "qb", "kb", "ob"):
        g[v + "_d"] = dt_in(v, [128, HL], F32)      # pp layout from host
    g["swv_d"] = dt_in("swv", [CH], F32)
    g["vb_d"] = dt_in("vb", [CH], F32)
    if mask_mode == "causal":
        g["tblk_d"] = dt_in("tblk", [128, 128], F32)
        g["tblkT_d"] = dt_in("tblkT", [128, 128], F32)
    if mask_mode == "general":
        g["mask_d"] = dt_in("mask", [T, T], F32)
        g["maskT_d"] = dt_in("maskT", [T, T], F32)
    g["rvr_d"] = dt_in("rvr", [4, 512], F32)

    g["outT_d"] = nc.dram_tensor("outT", [CH, T], F32,
                                 kind="ExternalOutput").ap()

    # internal DRAM (row/token order unless noted)
    g["kmx_in"] = nc.dram_tensor("kmx_in", [2, T], F32).ap()
    g["kmx_out"] = nc.dram_tensor("kmx_out", [2, T], F32,
                                  addr_space="Shared").ap()
    g["ym_in"] = nc.dram_tensor("ym_in", [T], F32).ap()       # pp order!
    g["ym_out"] = nc.dram_tensor("ym_out", [T], F32,
                                 addr_space="Shared").ap()
    g["warm_in"] = nc.dram_tensor("warm_in", [16], F32).ap()
    g["warm_out"] = nc.dram_tensor("warm_out", [16], F32,
                                   addr_space="Shared").ap()
    g["acc_in"] = nc.dram_tensor("acc_in", [T], F32).ap()
    g["acc_out"] = nc.dram_tensor("acc_out", [T], F32,
                                  addr_space="Shared").ap()
    g["ssel_b"] = nc.dram_tensor("ssel_b", [T], BF16).ap()
    g["flg_b"] = nc.dram_tensor("flg_b", [T], I32).ap()
    g["rs4_b"] = nc.dram_tensor("rs4_b", [T], F32).ap()
    g["rs8_b"] = nc.dram_tensor("rs8_b", [T], F32).ap()
    g["svm_b"] = nc.dram_tensor("svm_b", [NT * CH], F32).ap()
    g["sv_b"] = nc.dram_tensor("sv_b", [CH], F32).ap()
    g["z_b"] = nc.dram_tensor("z_b", [2, T], F32).ap()
    g["rz_b"] = nc.dram_tensor("rz_b", [2, T], F32).ap()
    g["cmu_b"] = nc.dram_tensor("cmu_b", [2, T], F32).ap()
    g["rsc_b"] = nc.dram_tensor("rsc_b", [T], F32).ap()
    g["sc_b"] = nc.dram_tensor("sc_b", [T], F32).ap()
    g["cmx_in"] = nc.dram_tensor("cmx_in", [T], F32).ap()
    g["cmx_out"] = nc.dram_tensor("cmx_out", [T], F32,
                                  addr_space="Shared").ap()
    g["ag_in0"] = nc.dram_tensor("ag_in0", [128 * T], BF16).ap()
    g["ag_out0"] = nc.dram_tensor("ag_out0", [NCORES, 128 * T], BF16,
                                  addr_space="Shared").ap()
    g["ag_in1"] = nc.dram_tensor("ag_in1", [128 * T], BF16).ap()
    g["ag_out1"] = nc.dram_tensor("ag_out1", [NCORES, 128 * T], BF16,
                                  addr_space="Shared").ap()
    g["rg"] = [list(range(NCORES))]

    with tile.TileContext(nc) as tc:
        _body(nc, tc, g)
    nc.compile()
    return nc


def _body(nc, tc, g):
    rg = g["rg"]
    causal = g["mode"] == "causal"
    general = g["mode"] == "general"

    def pool(name, bufs=1, space="SBUF"):
        cm = tc.tile_pool(name=name, bufs=bufs, space=space)
        return cm, cm.__enter__()

    # ---------------- persistent pool ----------------
    per_cm, per = pool("per")
    sx_pp = per.tile([128, NT], F32, tag="sxpp")
    nc.sync.dma_start(sx_pp[:], g["sxpp_d"][:])

    def load_pp(dram_pp, tag):
        t_ = per.tile([128, HL], F32, tag=tag)
        nc.sync.dma_start(t_[:], dram_pp[:])
        return t_

    swq_pp = load_pp(g["swq_d"], "swq")
    swk_pp = load_pp(g["swk_d"], "swk")
    swo_pp = load_pp(g["swo_d"], "swo")
    qb_pp = load_pp(g["qb_d"], "qb")
    kb_pp = load_pp(g["kb_d"], "kb")
    ob_pp = load_pp(g["ob_d"], "ob")
    if causal:
        tblk = per.tile([128, 128], F32, tag="tblk")
        tblkT = per.tile([128, 128], F32, tag="tblkT")
        nc.sync.dma_start(tblk[:], g["tblk_d"][:])
        nc.sync.dma_start(tblkT[:], g["tblkT_d"][:])
    ones_sb = per.tile([128, 1], BF16, tag="ones")
    nc.vector.memset(ones_sb[:], 1.0)
    rndc_pp = per.tile([128, 1], F32, tag="rndcpp")
    nc.vector.memset(rndc_pp[:], RND_C)
    swv_rep = per.tile([128, CH], F32, tag="swvrep")
    vb_rep = per.tile([128, CH], F32, tag="vbrep")
    nc.sync.dma_start(swv_rep[:],
                      g["swv_d"].rearrange("(a c) -> a c", a=1)
                      .to_broadcast([128, CH]))
    nc.sync.dma_start(vb_rep[:],
                      g["vb_d"].rearrange("(a c) -> a c", a=1)
                      .to_broadcast([128, CH]))

    qT = [per.tile([128, T], BF16, tag=f"qT{h}", name=f"qT{h}")
          for h in range(HL)]
    k2 = [per.tile([128, T], BF16, tag=f"k2{h}", name=f"k2{h}")
          for h in range(HL)]
    vqi = per.tile([128, NT * CH], BF16, tag="vqi")
    sv_pp = per.tile([128, HL], F32, tag="svpp")
    sy_pp = per.tile([128, NT], F32, tag="sypp")
    rsy_pp = per.tile([128, NT], F32, tag="rsypp")
    kmx4 = per.tile([4, 512], F32, tag="kmx4")
    s8r = per.tile([4, 512], F32, tag="s8r")
    s4r = per.tile([4, 512], F32, tag="s4r")
    rs8r = per.tile([4, 512], F32, tag="rs8r")
    rs4r = per.tile([4, 512], F32, tag="rs4r")
    wo_sb = per.tile([128, NE * CH], BF16, tag="wosb")

    # warm up the exp table set early (one-time ~2.7us load)
    wex = per.tile([1, 8], F32, tag="wex")
    nc.scalar.activation(wex[:], sx_pp[0:1, 0:8], ACTF.Exp)

    # ---------------- wA: yv/v1i (live to gap end) ----------------
    wA_cm, wA = pool("wA")
    yv = wA.tile([128, NT * CH], F32, tag="yv")
    v1i = wA.tile([128, NT * CH], BF16, tag="v1i")
    ym_pp = wA.tile([128, NT], F32, tag="ympp")
    kTx = [wA.tile([128, T], F32, tag=f"kTx{h}", name=f"kTx{h}")
           for h in range(HL)]
    kTr = [wA.tile([128, T], BF16, tag=f"kTr{h}", name=f"kTr{h}")
           for h in range(HL)]

    # ---------------- wX: xq + resident weights (die after proj) --------
    wX_cm, wX = pool("wX")
    xq = wX.tile([128, NE * T], BF16, tag="xq")
    wk_sb = wX.tile([128, NE * CH], BF16, tag="wksb")
    wv_sb = wX.tile([128, NE * CH], BF16, tag="wvsb")
    sx_rep = wX.tile([128, T], F32, tag="sxrep")
    # queue plan: sync = xq 0-5,12-15 + sx_rep; gpsimd = xq 6-11 + wk +
    # wv; scalar = streamed wq tiles (inside the Q pass below)
    for et in range(6):
        nc.sync.dma_start(xq[:, et * T:(et + 1) * T],
                          g["xq_d"][et * 128:(et + 1) * 128, :])
    for et in range(6, 12):
        nc.gpsimd.dma_start(xq[:, et * T:(et + 1) * T],
                            g["xq_d"][et * 128:(et + 1) * 128, :])
    for et in range(NE):
        nc.gpsimd.dma_start(wk_sb[:, et * CH:(et + 1) * CH],
                            g["wk_d"][et * 128:(et + 1) * 128, :])
    for et in range(NE):
        nc.gpsimd.dma_start(wv_sb[:, et * CH:(et + 1) * CH],
                            g["wv_d"][et * 128:(et + 1) * 128, :])
    nc.sync.dma_start(sx_rep[:],
                      g["sx_d"].rearrange("(a t) -> a t", a=1)
                      .to_broadcast([128, T]))

    # -------- Q/K projections (half-T PSUM) + V interleaved in K --------
    p1b_cm, p1b = pool("p1b", bufs=3)
    ps1_cm, ps1 = pool("ps1", space="PSUM")
    ps1v_cm, ps1v = pool("ps1v", space="PSUM")

    def v_block(j):
        pV = ps1v.tile([128, CH], F32, tag="pV", name=f"pV{j}")
        for et2 in range(NE):
            nc.tensor.matmul(pV[:],
                             xq[:, et2 * T + j * 128:et2 * T + (j + 1) * 128],
                             wv_sb[:, et2 * CH:(et2 + 1) * CH],
                             start=(et2 == 0), stop=(et2 == NE - 1))
        jsl = slice(j * CH, (j + 1) * CH)
        e3 = p1b.tile([128, CH], F32, tag="e3", bufs=2)
        nc.vector.tensor_scalar(e3[:], pV[:], sx_pp[:, j:j + 1],
                                None, op0=OP.mult)
        nc.vector.tensor_tensor(e3[:], e3[:], swv_rep[:], op=OP.mult)
        nc.vector.tensor_tensor(yv[:, jsl], e3[:], vb_rep[:], op=OP.add)
        nc.vector.tensor_reduce(ym_pp[:, j:j + 1], yv[:, jsl],
                                axis=AX.X, op=OP.max,
                                apply_absolute_value=True)

    ets_v1 = [e for e in range(NE) if e % 4 != 0]   # 12 slots in K half 1
    for proj in ("q", "k"):
        for thalf in range(2):
            tsl = slice(thalf * 1024, (thalf + 1) * 1024)
            pP = [ps1.tile([128, 1024], F32, tag=f"pP{o}",
                           name=f"pP{o}_{proj}{thalf}") for o in range(2)]
            for et in range(NE):
                if proj == "q":
                    we = p1b.tile([128, CH], BF16, tag="wstream")
                    nc.scalar.dma_start(we[:],
                                        g["wq_d"][et * 128:(et + 1) * 128, :])
                    if thalf == 0 and et % 4 == 3:
                        xet = 12 + et // 4
                        nc.scalar.dma_start(
                            xq[:, xet * T:(xet + 1) * T],
                            g["xq_d"][xet * 128:(xet + 1) * 128, :])
                    wsrc = we
                else:
                    wsrc = wk_sb[:, et * CH:(et + 1) * CH]
                for o in range(2):
                    for n2 in range(2):
                        nc.tensor.matmul(
                            pP[o][:, n2 * 512:(n2 + 1) * 512],
                            wsrc[:, o * 128:(o + 1) * 128],
                            xq[:, et * T + thalf * 1024 + n2 * 512:
                               et * T + thalf * 1024 + (n2 + 1) * 512],
                            start=(et == 0), stop=(et == NE - 1))
                # V blocks: 4 late in K half 0, 12 spread over K half 1
                if proj == "k" and thalf == 0 and et >= 12:
                    v_block(et - 12)
                if proj == "k" and thalf == 1 and et in ets_v1:
                    v_block(4 + ets_v1.index(et))
            for o in range(2):
                e1 = p1b.tile([128, 1024], F32, tag="ev1", bufs=2)
                sw = swq_pp if proj == "q" else swk_pp
                bb = qb_pp if proj == "q" else kb_pp
                nc.scalar.activation(e1[:], pP[o][:], ACTF.Copy,
                                     scale=sw[:, o:o + 1])
                nc.vector.tensor_tensor(e1[:], e1[:], sx_rep[:, tsl],
                                        op=OP.mult)
                if proj == "q":
                    nc.vector.tensor_scalar(qT[o][:, tsl], e1[:],
                                            bb[:, o:o + 1], SCALING,
                                            op0=OP.add, op1=OP.mult)
                else:
                    nc.vector.tensor_scalar(kTx[o][:, tsl], e1[:],
                                            bb[:, o:o + 1], None, op0=OP.add)
                    nc.scalar.activation(kTr[o][:, tsl], e1[:],
                                         ACTF.Identity, bias=bb[:, o:o + 1])
    ps1v_cm.__exit__(None, None, None)
    ps1_cm.__exit__(None, None, None)

    # ym AR first (absorbs skew, hides under pass 1); pp order is fine
    # because elementwise max is layout-agnostic if all cores agree
    nc.sync.dma_start(g["ym_in"].rearrange("(p j) -> p j", p=128), ym_pp[:])
    nc.gpsimd.collective_compute("AllReduce", OP.max,
                                 ins=[g["ym_in"][:]], outs=[g["ym_out"][:]],
                                 replica_groups=rg)
    # kmax AR
    kmxs = p1b.tile([128, T], F32, tag="kmxs", bufs=1)
    for hh in range(HL):
        nc.gpsimd.partition_all_reduce(kmxs[:], kTx[hh][:], 128, RED.absmax)
        nc.sync.dma_start(g["kmx_in"][hh, :].rearrange("(a t) -> a t", a=1),
                          kmxs[0:1, :])
    nc.gpsimd.collective_compute("AllReduce", OP.max,
                                 ins=[g["kmx_in"][:]], outs=[g["kmx_out"][:]],
                                 replica_groups=rg)
    p1b_cm.__exit__(None, None, None)
    wX_cm.__exit__(None, None, None)

    # prefetch o-proj weights during pass 1 (sync/scalar queues)
    for et in range(NE):
        (nc.sync if et % 2 == 0 else nc.scalar).dma_start(
            wo_sb[:, et * CH:(et + 1) * CH],
            g["wo_d"][et * 128:(et + 1) * 128, :])

    # wB: k-int tiles + scale replicas (allocated in xq's freed space)
    wB_cm, wB = pool("wB")
    k4i = [wB.tile([128, T], BF16, tag=f"k4i{h}", name=f"k4i{h}")
           for h in range(HL)]
    k8i = [wB.tile([128, T], BF16, tag=f"k8i{h}", name=f"k8i{h}")
           for h in range(HL)]
    rs4_rep = wB.tile([128, T], F32, tag="rs4rep")
    rs8_rep = wB.tile([128, T], F32, tag="rs8rep")

    # ---------------- pass 1 (i outer, heads inner) ----------------
    p5_cm, p5 = pool("p5", bufs=3)
    p5m_cm, p5m = pool("p5m", bufs=2)
    psS_cm, psS_p = pool("psS", bufs=1, space="PSUM")
    psA_cm, psA_p = pool("psA", space="PSUM")
    pA = [psA_p.tile([1, 512], F32, tag=f"pA{n}", name=f"pA{n}")
          for n in range(4)]
    first_wr = [True] * 4
    vmx = p5.tile([128, 8 * CH], F32, tag="vmx", bufs=1)

    def vquant_chunk(i):
        isl = slice(i * CH, (i + 1) * CH)
        d1 = p5.tile([128, CH], F32, tag="d1", bufs=3)
        nc.vector.tensor_scalar(d1[:], yv[:, isl], rsy_pp[:, i:i + 1],
                                RND_C, op0=OP.mult, op1=OP.add)
        nc.vector.tensor_scalar(v1i[:, isl], d1[:], RND_C, None,
                                op0=OP.subtract)
        nc.vector.tensor_scalar(yv[:, isl], v1i[:, isl], sy_pp[:, i:i + 1],
                                None, op0=OP.mult)

    def kint_quant(h, rrep, dst):
        # dst = round(kTx[h] * rrep) as bf16 ints (runs on idle gpsimd)
        t8 = p5.tile([128, T], F32, tag="kq8", bufs=2)
        nc.gpsimd.tensor_tensor(t8[:], kTx[h][:], rrep[:], op=OP.mult)
        nc.gpsimd.tensor_scalar(t8[:], t8[:], RND_C, None, op0=OP.add)
        nc.gpsimd.tensor_scalar(dst[:], t8[:], RND_C, None,
                                op0=OP.subtract)

    for i in range(NT):
        c_cols = (i + 1) * 128 if causal else T
        nch = _cdiv(c_cols, 512)
        diag_n, diag_off = (i * 128) // 512, (i * 128) % 512
        if general:
            mrow = p5m.tile([128, T], F32, tag="mrow")
            nc.sync.dma_start(mrow[:],
                              g["mask_d"][i * 128:(i + 1) * 128, :])
        for h in range(HL):
            zz = p5.tile([128, 4], F32, tag=f"zz{h}")
            pp = []
            for n in range(nch):
                w = min(512, c_cols - n * 512)
                psS = psS_p.tile([128, 512], F32, tag=f"pS{h}", bufs=2,
                                 name=f"pS_{h}_{i}_{n}")
                nc.tensor.matmul(psS[:, :w],
                                 qT[h][:, i * 128:(i + 1) * 128],
                                 kTr[h][:, n * 512:n * 512 + w],
                                 start=True, stop=True)
                if causal and n == diag_n:
                    nc.vector.tensor_tensor(psS[:, diag_off:diag_off + 128],
                                            psS[:, diag_off:diag_off + 128],
                                            tblk[:], op=OP.add)
                elif general:
                    nc.vector.tensor_tensor(psS[:, :w], psS[:, :w],
                                            mrow[:, n * 512:n * 512 + w],
                                            op=OP.add)
                p1t = p5.tile([128, 512], BF16, tag=f"p1t{h}", bufs=4,
                              name=f"p1t_{h}_{i}_{n}")
                nc.scalar.activation(p1t[:, :w], psS[:, :w], ACTF.Exp,
                                     bias=0.0, scale=1.0,
                                     accum_out=zz[:, n:n + 1])
                if w < 512:
                    nc.vector.memset(p1t[:, w:], 0.0)
                pp.append(p1t)
            z = p5.tile([128, 1], F32, tag=f"z{h}")
            if nch == 1:
                nc.vector.tensor_copy(z[:], zz[:, 0:1])
            else:
                nc.vector.tensor_reduce(z[:], zz[:, :nch], axis=AX.X,
                                        op=OP.add)
            rz = p5.tile([128, 1], BF16, tag=f"rz{h}")
            with nc.allow_low_precision(reason="bf16 matmul feed"):
                nc.vector.reciprocal(rz[:], z[:])
            for n in range(nch):
                nc.tensor.matmul(pA[n][:], rz[:], pp[n][:],
                                 start=first_wr[n],
                                 stop=(i == NT - 1 and h == HL - 1))
                first_wr[n] = False

        # ---- i-driven slack work ----
        if i == 7:
            # quant-scale prep (ym/kmx ARs done by now)
            ymf = p5.tile([128, NT], F32, tag="ymf", bufs=1)
            nc.sync.dma_start(
                ymf[:], g["ym_out"].rearrange("(p j) -> p j", p=128))
            nc.vector.tensor_scalar(sy_pp[:], ymf[:], EPS, 1.0 / Q8,
                                    op0=OP.max, op1=OP.mult)
            nc.vector.reciprocal(rsy_pp[:], sy_pp[:])
        if i == 10:
            # k-quant scales (kmx AR done by now)
            nc.sync.dma_start(
                kmx4[:],
                g["kmx_out"][0, :].rearrange("(r s) -> r s", r=4))
            km2 = p5.tile([4, 512], F32, tag="km2", bufs=1)
            nc.sync.dma_start(
                km2[:],
                g["kmx_out"][1, :].rearrange("(r s) -> r s", r=4))
            nc.vector.tensor_tensor(kmx4[:], kmx4[:], km2[:],
                                    op=OP.max)
            nc.vector.tensor_scalar(s8r[:], kmx4[:], 1.0 / Q8, EPS,
                                    op0=OP.mult, op1=OP.max)
            nc.vector.tensor_scalar(s4r[:], kmx4[:], EPS, 1.0 / Q4,
                                    op0=OP.max, op1=OP.mult)
            nc.vector.reciprocal(rs8r[:], s8r[:])
            nc.vector.reciprocal(rs4r[:], s4r[:])
            # broadcast reciprocal-scale rows for the kint precompute
            nc.scalar.dma_start(g["rs4_b"].rearrange("(r s) -> r s", r=4),
                                rs4r[:])
            nc.scalar.dma_start(g["rs8_b"].rearrange("(r s) -> r s", r=4),
                                rs8r[:])
            nc.scalar.dma_start(rs4_rep[:],
                                g["rs4_b"].rearrange("(a t) -> a t", a=1)
                                .to_broadcast([128, T]))
            nc.scalar.dma_start(rs8_rep[:],
                                g["rs8_b"].rearrange("(a t) -> a t", a=1)
                                .to_broadcast([128, T]))
        if 8 <= i <= 15:
            vquant_chunk(2 * (i - 8))
            vquant_chunk(2 * (i - 8) + 1)
        if i == 11:
            kint_quant(0, rs4_rep, k4i[0])
        if i == 12:
            kint_quant(0, rs8_rep, k8i[0])
        if i == 13:
            kint_quant(1, rs4_rep, k4i[1])
        if i == 14:
            kint_quant(1, rs8_rep, k8i[1])
        if i == 12:
            nc.gpsimd.partition_all_reduce(vmx[:], yv[:, :8 * CH],
                                           128, RED.absmax)
            nc.sync.dma_start(
                g["svm_b"][0:8 * CH].rearrange("(a c) -> a c", a=1),
                vmx[0:1, :])
        if i == 14:
            # warmup AR to absorb inter-core skew before the acc AR
            wtl = p5.tile([1, 16], F32, tag="wtl", bufs=1)
            nc.vector.memset(wtl[:], 1.0)
            nc.sync.dma_start(
                g["warm_in"].rearrange("(a t) -> a t", a=1), wtl[:])
            nc.gpsimd.collective_compute(
                "AllReduce", OP.max,
                ins=[g["warm_in"][:]], outs=[g["warm_out"][:]],
                replica_groups=rg)

    # ---- end i loop: single acc AR for both heads ----
    accs = p5.tile([1, T], F32, tag="accs", bufs=1)
    for n in range(4):
        nc.vector.tensor_copy(accs[:, n * 512:(n + 1) * 512], pA[n][:])
    nc.sync.dma_start(g["acc_in"].rearrange("(a t) -> a t", a=1), accs[:])
    nc.gpsimd.collective_compute("AllReduce", OP.add,
                                 ins=[g["acc_in"][:]], outs=[g["acc_out"][:]],
                                 replica_groups=rg)
    nc.gpsimd.partition_all_reduce(vmx[:], yv[:, 8 * CH:],
                                   128, RED.absmax)
    nc.sync.dma_start(
        g["svm_b"][8 * CH:].rearrange("(a c) -> a c", a=1),
        vmx[0:1, :])
    psA_cm.__exit__(None, None, None)
    psS_cm.__exit__(None, None, None)
    p5m_cm.__exit__(None, None, None)
    p5_cm.__exit__(None, None, None)

    # -------- sv scales + vqi (hide under acc AR) + topk + k2 --------
    p6_cm, p6 = pool("p6")
    sv16 = p6.tile([16, CH], F32, tag="sv16")
    nc.sync.dma_start(sv16[:],
                      g["svm_b"].rearrange("(j c) -> j c", j=16))
    sv16r = p6.tile([16, CH], F32, tag="sv16r")
    nc.gpsimd.partition_all_reduce(sv16r[:], sv16[:], 16, RED.max)
    sv_row = p6.tile([1, CH], F32, tag="svrow")
    nc.vector.tensor_scalar(sv_row[:], sv16r[0:1, :], EPS, 1.0 / Q8,
                            op0=OP.max, op1=OP.mult)
    rsv_row = p6.tile([1, CH], F32, tag="rsvrow")
    nc.vector.reciprocal(rsv_row[:], sv_row[:])
    rsv_rep = p6.tile([128, CH], F32, tag="rsvrep")
    nc.gpsimd.partition_broadcast(rsv_rep[:], rsv_row[:])
    nc.sync.dma_start(g["sv_b"].rearrange("(a c) -> a c", a=1), sv_row[:])
    nc.sync.dma_start(sv_pp[:],
                      g["sv_b"].rearrange("(h p) -> p h", p=128))
    # vqi quant on gpsimd: hides under the acc AR / early pass 2
    for j in range(NT):
        jsl = slice(j * CH, (j + 1) * CH)
        m1 = p6.tile([128, CH], F32, tag="m1", bufs=3)
        nc.gpsimd.tensor_tensor(m1[:], yv[:, jsl], rsv_rep[:], op=OP.mult)
        nc.gpsimd.tensor_scalar(m1[:], m1[:], RND_C, None, op0=OP.add)
        nc.gpsimd.tensor_scalar(vqi[:, jsl], m1[:], RND_C, None,
                                op0=OP.subtract)

    acc4 = p6.tile([4, 512], F32, tag="acc4")
    nc.sync.dma_start(acc4[:], g["acc_out"].rearrange("(r s) -> r s", r=4))
    rvr = p6.tile([4, 512], F32, tag="rvr")
    nc.sync.dma_start(rvr[:], g["rvr_d"][:])
    nc.vector.tensor_tensor(acc4[:], acc4[:], rvr[:], op=OP.mult)
    tkw = p6.tile([4, 512], F32, tag="tkw")
    ton = acc4[:]
    for k_on in range(0, K_TOP, 8):
        k_this = min(k_on + 8, K_TOP) - k_on
        mx8 = p6.tile([4, 8], F32, tag="mx8")
        nc.vector.max(out=mx8[:], in_=ton)
        if k_this < 8:
            nc.vector.memset(mx8[:, k_this:], 0)
        nc.vector.match_replace(out=tkw[:], in_to_replace=mx8[:],
                                in_values=ton, imm_value=0)
        ton = tkw[:]
    nc.vector.tensor_sub(out=tkw[:], in0=acc4[:], in1=tkw[:])
    flg4 = p6.tile([4, 512], I32, tag="flg4")
    nc.vector.tensor_scalar(flg4[:], tkw[:], 0.0, None, op0=OP.is_gt)
    nc.scalar.dma_start(g["flg_b"].rearrange("(r s) -> r s", r=4), flg4[:])
    nc.vector.copy_predicated(s4r[:], flg4[:], s8r[:])
    ssel4 = p6.tile([4, 512], BF16, tag="ssel4")
    nc.vector.tensor_copy(ssel4[:], s4r[:])
    nc.sync.dma_start(g["ssel_b"].rearrange("(r s) -> r s", r=4), ssel4[:])
    ssel_rep = p6.tile([128, T], BF16, tag="sselrep")
    flg_rep = p6.tile([128, T], I32, tag="flgrep")
    # chunked broadcasts + chunked k2 build so pass 2 starts on chunk 0
    for cn in range(4):
        csl = slice(cn * 512, (cn + 1) * 512)
        nc.scalar.dma_start(flg_rep[:, csl],
                            g["flg_b"][csl].rearrange("(a t) -> a t", a=1)
                            .to_broadcast([128, 512]))
        nc.sync.dma_start(ssel_rep[:, csl],
                          g["ssel_b"][csl].rearrange("(a t) -> a t", a=1)
                          .to_broadcast([128, 512]))
    for hh in range(HL):
        for cn in range(4):
            csl = slice(cn * 512, (cn + 1) * 512)
            nc.vector.copy_predicated(k4i[hh][:, csl], flg_rep[:, csl],
                                      k8i[hh][:, csl])
            nc.vector.tensor_tensor(k2[hh][:, csl], k4i[hh][:, csl],
                                    ssel_rep[:, csl], op=OP.mult)
    p6_cm.__exit__(None, None, None)
    wB_cm.__exit__(None, None, None)
    wA_cm.__exit__(None, None, None)

    # ---------------- pass 2 (flipped, j-outer half rows) ----------------
    p7_cm, p7 = pool("p7", bufs=3)
    p7m_cm, p7m = pool("p7m", bufs=2)
    ps2S_cm, ps2S = pool("ps2s", bufs=2, space="PSUM")
    ps2C_cm, ps2C = pool("ps2c", space="PSUM")
    ps2Z_cm, ps2Z = pool("ps2z", space="PSUM")
    ctxU = [p7.tile([128, T], F32, tag=f"ctxU{h}", name=f"ctxU{h}", bufs=1)
            for h in range(HL)]
    rz_rep = [p7.tile([128, T], F32, tag="rzrep", name=f"rzrep{h}", bufs=1)
              for h in range(HL)]
    ctxb = [p7.tile([128, T], BF16, tag=f"ctxb{h}", name=f"ctxb{h}", bufs=1)
            for h in range(HL)]
    cmup = [p7.tile([128, T], F32, tag=f"cmup{h}", name=f"cmup{h}", bufs=1)
            for h in range(HL)]
    cm4 = [p7.tile([4, 512], F32, tag=f"cm4{h}", name=f"cm4{h}", bufs=1)
           for h in range(HL)]

    for h in range(HL):
        for half in range(2):
            tlo = half * 1024
            jmax = (8 + 8 * half) if causal else NT
            psC = [ps2C.tile([128, 512], F32, tag=f"pC{s}",
                             name=f"pC{s}_{h}_{half}") for s in range(2)]
            psZ = [ps2Z.tile([1, 512], F32, tag=f"pZ{s}",
                             name=f"pZ{s}_{h}_{half}") for s in range(2)]
            for j in range(jmax):
                off = max(0, j * 128 - tlo) if causal else 0
                psSX = ps2S.tile([128, 1024], F32, tag="pSX",
                                 name=f"pSX{h}_{half}_{j}")
                for s2 in range(2):
                    lo2 = max(off, s2 * 512)
                    if lo2 >= (s2 + 1) * 512:
                        continue
                    nc.tensor.matmul(psSX[:, lo2:(s2 + 1) * 512],
                                     k2[h][:, j * 128:(j + 1) * 128],
                                     qT[h][:, tlo + lo2:
                                           tlo + (s2 + 1) * 512],
                                     start=True, stop=True)
                if causal and j * 128 >= tlo:
                    nc.vector.tensor_tensor(psSX[:, off:off + 128],
                                            psSX[:, off:off + 128],
                                            tblkT[:], op=OP.add)
                elif general:
                    mrowT = p7m.tile([128, 1024], F32, tag="mrowT")
                    nc.sync.dma_start(
                        mrowT[:], g["maskT_d"][j * 128:(j + 1) * 128,
                                               tlo:tlo + 1024])
                    nc.vector.tensor_tensor(psSX[:], psSX[:], mrowT[:],
                                            op=OP.add)
                p2t = p7.tile([128, 1024], BF16, tag="p2t", bufs=3,
                              name=f"p2t{h}_{half}_{j}")
                if off > 0:
                    nc.vector.memset(p2t[:, :off], 0.0)
                nc.scalar.activation(p2t[:, off:], psSX[:, off:], ACTF.Exp,
                                     bias=0.0, scale=1.0)
                for s in range(2):
                    n = 2 * half + s
                    jm = min(4 * n + 4, jmax) if causal else jmax
                    if j >= jm:
                        continue
                    ssl = slice(s * 512, (s + 1) * 512)
                    nc.tensor.matmul(psZ[s][:], ones_sb[:], p2t[:, ssl],
                                     start=(j == 0), stop=(j == jm - 1))
                    nc.tensor.matmul(psC[s][:],
                                     vqi[:, j * CH + h * 128:
                                         j * CH + (h + 1) * 128],
                                     p2t[:, ssl],
                                     start=(j == 0), stop=(j == jm - 1))
            for s in range(2):
                nsl = slice(tlo + s * 512, tlo + (s + 1) * 512)
                nc.vector.tensor_scalar(ctxU[h][:, nsl], psC[s][:],
                                        sv_pp[:, h:h + 1], None, op0=OP.mult)
                zst = p7.tile([1, 512], F32, tag="zst", bufs=2,
                              name=f"zst{h}_{half}_{s}")
                nc.vector.tensor_copy(zst[:], psZ[s][:])
                nc.sync.dma_start(
                    g["z_b"][h, tlo + s * 512:tlo + (s + 1) * 512]
                    .rearrange("(a t) -> a t", a=1), zst[:])
            # per-half unnormalized channel-absmax (hides under next half)
            nc.gpsimd.partition_all_reduce(cmup[h][:, tlo:tlo + 1024],
                                           ctxU[h][:, tlo:tlo + 1024],
                                           128, RED.absmax)
        nc.sync.dma_start(g["cmu_b"][h, :].rearrange("(a t) -> a t", a=1),
                          cmup[h][0:1, :])
        # z -> [4,512] recip -> rz row -> broadcast prefetch
        z4 = p7.tile([4, 512], F32, tag="z4", name=f"z4_{h}", bufs=2)
        nc.sync.dma_start(z4[:],
                          g["z_b"][h, :].rearrange("(r s) -> r s", r=4))
        rz4 = p7.tile([4, 512], F32, tag="rz4", name=f"rz4_{h}", bufs=2)
        nc.vector.reciprocal(rz4[:], z4[:])
        # cmax row for this head first: the cmx AR input must be ready
        # BEFORE ag_in1 so the scheduler runs the AR ahead of AG1
        nc.sync.dma_start(cm4[h][:],
                          g["cmu_b"][h, :].rearrange("(r s) -> r s", r=4))
        nc.vector.tensor_tensor(cm4[h][:], cm4[h][:], rz4[:], op=OP.mult)
        if h == 1:
            cmr = p7.tile([4, 512], F32, tag="cmr", bufs=1)
            nc.vector.tensor_tensor(cmr[:], cm4[0][:], cm4[1][:],
                                    op=OP.max)
            nc.sync.dma_start(g["cmx_in"].rearrange("(r s) -> r s", r=4),
                              cmr[:])
        nc.sync.dma_start(g["rz_b"][h, :].rearrange("(r s) -> r s", r=4),
                          rz4[:])
        nc.sync.dma_start(
            rz_rep[h][:], g["rz_b"][h, :].rearrange("(a t) -> a t", a=1)
            .to_broadcast([128, T]))
        nc.vector.tensor_tensor(ctxU[h][:], ctxU[h][:], rz_rep[h][:],
                                op=OP.mult)
        # raw bf16 ctx for the AllGather
        nc.vector.tensor_copy(ctxb[h][:], ctxU[h][:])
        ag_in = g["ag_in0"] if h == 0 else g["ag_in1"]
        nc.sync.dma_start(ag_in.rearrange("(p t) -> p t", p=128),
                          ctxb[h][:])
        if h == 0:
            nc.gpsimd.collective_compute(
                "AllGather", OP.bypass,
                ins=[g["ag_in0"][:]], outs=[g["ag_out0"][:]],
                replica_groups=rg)
    # cmax AR (before AG1 so o-proj scales arrive ASAP)
    nc.gpsimd.collective_compute("AllReduce", OP.max,
                                 ins=[g["cmx_in"][:]], outs=[g["cmx_out"][:]],
                                 replica_groups=rg)
    nc.gpsimd.collective_compute("AllGather", OP.bypass,
                                 ins=[g["ag_in1"][:]], outs=[g["ag_out1"][:]],
                                 replica_groups=rg)
    ps2Z_cm.__exit__(None, None, None)
    ps2C_cm.__exit__(None, None, None)
    ps2S_cm.__exit__(None, None, None)
    p7m_cm.__exit__(None, None, None)
    p7_cm.__exit__(None, None, None)

    # -------- gathered-tile loads + quant scales --------
    ps9_cm, ps9 = pool("ps9", space="PSUM")
    p9_cm, p9 = pool("p9", bufs=2)
    ag30 = g["ag_out0"].rearrange("c (p t) -> c p t", p=128)
    ag31 = g["ag_out1"].rearrange("c (p t) -> c p t", p=128)
    # head-0 tiles: resident, loaded on scalar as soon as AG0 lands
    # (hides under the cmax AR); head-1 tiles stream via gpsimd queue
    ct0 = [p9.tile([128, T], BF16, tag=f"ct0_{c2}", bufs=1,
                   name=f"ct0_{c2}") for c2 in range(NCORES)]
    for c2 in range(NCORES):
        nc.scalar.dma_start(ct0[c2][:], ag30[c2])

    cmx4 = p9.tile([4, 512], F32, tag="cmx4", bufs=1)
    nc.sync.dma_start(cmx4[:],
                      g["cmx_out"].rearrange("(r s) -> r s", r=4))
    sc4 = p9.tile([4, 512], F32, tag="sc4", bufs=1)
    nc.vector.tensor_scalar(sc4[:], cmx4[:], EPS, 1.0 / Q8,
                            op0=OP.max, op1=OP.mult)
    nc.sync.dma_start(g["sc_b"].rearrange("(r s) -> r s", r=4), sc4[:])
    rsc4 = p9.tile([4, 512], F32, tag="rsc4", bufs=1)
    nc.vector.reciprocal(rsc4[:], sc4[:])
    nc.scalar.dma_start(g["rsc_b"].rearrange("(r s) -> r s", r=4), rsc4[:])
    rsc_rep = p9.tile([128, T], F32, tag="rscrep", bufs=1)
    nc.scalar.dma_start(rsc_rep[:],
                        g["rsc_b"].rearrange("(a t) -> a t", a=1)
                        .to_broadcast([128, T]))
    sc_rep = p9.tile([128, T], F32, tag="screp", bufs=1)
    nc.sync.dma_start(sc_rep[:],
                      g["sc_b"].rearrange("(a t) -> a t", a=1)
                      .to_broadcast([128, T]))

    # -------- output projection over gathered tiles --------
    pO = [ps9.tile([128, T], F32, tag=f"pO{o}", name=f"pO{o}")
          for o in range(2)]
    order = [(c2, hh) for hh in range(HL) for c2 in range(NCORES)]
    nord = len(order)
    for idx, (c2, hh) in enumerate(order):
        if hh == 0:
            ct = ct0[c2]
        else:
            ct = p9.tile([128, T], BF16, tag="ct1", bufs=4,
                         name=f"ct1_{c2}")
            nc.gpsimd.dma_start(ct[:], ag31[c2])
        ctf = p9.tile([128, T], F32, tag="ctf", bufs=3,
                      name=f"ctf{hh}_{c2}")
        nc.vector.tensor_tensor(ctf[:], ct[:], rsc_rep[:], op=OP.mult)
        nc.vector.tensor_scalar(ctf[:], ctf[:], RND_C, None, op0=OP.add)
        cti = p9.tile([128, T], BF16, tag="cti", bufs=4,
                      name=f"cti{hh}_{c2}")
        nc.vector.tensor_scalar(cti[:], ctf[:], RND_C, None,
                                op0=OP.subtract)
        gidx = 2 * c2 + hh
        last = idx == nord - 1
        for o in range(2):
            wsl = slice(gidx * CH + o * 128, gidx * CH + (o + 1) * 128)
            for nn in range(4):
                nsl = slice(nn * 512, (nn + 1) * 512)
                nc.tensor.matmul(pO[o][:, nsl], wo_sb[:, wsl], cti[:, nsl],
                                 start=(idx == 0), stop=last)
                if last:
                    # per-chunk epilogue overlaps the remaining matmuls
                    f1 = p9.tile([128, 512], F32, tag="f1", bufs=4,
                                 name=f"f1{o}_{nn}")
                    nc.vector.tensor_scalar(f1[:], pO[o][:, nsl],
                                            swo_pp[:, o:o + 1], None,
                                            op0=OP.mult)
                    nc.vector.tensor_tensor(f1[:], f1[:], sc_rep[:, nsl],
                                            op=OP.mult)
                    nc.vector.tensor_scalar(f1[:], f1[:], ob_pp[:, o:o + 1],
                                            None, op0=OP.add)
                    nc.sync.dma_start(g["outT_d"][o * 128:(o + 1) * 128,
                                                  nsl], f1[:])
    p9_cm.__exit__(None, None, None)
    ps9_cm.__exit__(None, None, None)
    per_cm.__exit__(None, None, None)


# ==================== host side ====================

_CACHE = {}


def _get_nc(mode):
    if mode not in _CACHE:
        _CACHE[mode] = build(mode)
    return _CACHE[mode]


def _quant_w(w):
    amax = np.max(np.abs(w), axis=-1, keepdims=True)
    s = np.maximum(amax, np.float32(EPS)) / np.float32(Q8)
    wi = np.round((w / s).astype(np.float32))
    return wi, s[:, 0].astype(np.float32)


def _pp(vec, cols):
    # [cols*128] token/channel-order vector -> [128, cols] per-partition
    return np.ascontiguousarray(np.asarray(vec, np.float32)
                                .reshape(cols, 128).T)


def kernel(hidden_states, attention_mask, q_w, q_b, k_w, k_b, v_w, v_b,
           o_w, o_b, num_heads):
    hidden_states = np.asarray(hidden_states, dtype=np.float32)
    attention_mask = np.asarray(attention_mask, dtype=np.float32)
    assert int(num_heads) == H
    B, T_, E_ = hidden_states.shape
    assert (B, T_, E_) == (1, T, E)

    x = np.ascontiguousarray(hidden_states[0])        # [T, E]
    sx = (np.maximum(np.abs(x).max(axis=-1, keepdims=True),
                     np.float32(EPS)) / np.float32(Q8)).astype(np.float32)
    xi = np.round(x / sx)                             # ints, bf16-exact
    xqT = np.ascontiguousarray(xi.T).astype(ml_dtypes.bfloat16)   # [E, T]

    causal_ref = np.triu(np.full((T, T), np.float32(NEG), np.float32), k=1)
    mfull = np.ascontiguousarray(attention_mask[0, 0])
    if np.array_equal(mfull, causal_ref):
        mode = "causal"
    elif not mfull.any():
        mode = "zero"
    else:
        mode = "general"

    nc = _get_nc(mode)

    wqi, sq = _quant_w(np.asarray(q_w, np.float32))
    wki, sk = _quant_w(np.asarray(k_w, np.float32))
    wvi, sv = _quant_w(np.asarray(v_w, np.float32))
    woi, so = _quant_w(np.asarray(o_w, np.float32))

    tblk = np.triu(np.full((128, 128), np.float32(NEG), np.float32), k=1)
    tblkT = np.ascontiguousarray(tblk.T)
    rowvec = np.float32(T) - np.arange(T, dtype=np.float32)
    rvr = (np.float32(1.0) / rowvec).reshape(4, 512).astype(np.float32)

    in_maps = []
    for c in range(NCORES):
        ch = slice(c * CH, (c + 1) * CH)
        woT = np.ascontiguousarray(woi[ch, :].T)      # [E, CH]
        im = dict(
            xq=xqT,
            sx=np.ascontiguousarray(sx[:, 0]),
            sxpp=_pp(sx[:, 0], NT),
            wq=np.ascontiguousarray(wqi[ch, :].T).astype(ml_dtypes.bfloat16),
            wk=np.ascontiguousarray(wki[ch, :].T).astype(ml_dtypes.bfloat16),
            wv=np.ascontiguousarray(wvi[ch, :].T).astype(ml_dtypes.bfloat16),
            wo=woT.astype(ml_dtypes.bfloat16),
            swq=_pp(sq[ch], HL),
            swk=_pp(sk[ch], HL),
            swo=_pp(so[ch], HL),
            swv=np.ascontiguousarray(sv[ch]),
            qb=_pp(np.asarray(q_b, np.float32)[ch], HL),
            kb=_pp(np.asarray(k_b, np.float32)[ch], HL),
            ob=_pp(np.asarray(o_b, np.float32)[ch], HL),
            vb=np.ascontiguousarray(np.asarray(v_b, np.float32)[ch]),
            rvr=rvr,
        )
        if mode == "causal":
            im["tblk"] = tblk
            im["tblkT"] = tblkT
        if mode == "general":
            im["mask"] = mfull
            im["maskT"] = np.ascontiguousarray(mfull.T)
        in_maps.append(im)

    res = run_bass_kernel_spmd(nc, in_maps, list(range(NCORES)))
    kernel.last_results = res.results
    out = np.empty((T, E), dtype=np.float32)
    for c in range(NCORES):
        out[:, c * CH:(c + 1) * CH] = res.results[c]["outT"].T
    return out.reshape(1, T, E)


# revision 21
# speedup vs baseline: 1.4928x; 1.4928x over previous
"""CalScaleOPTAttention on 8 TRN2 NeuronCores — v3.0.

Sharding: heads across cores (2 heads / core, 256 channels each).

v3 changes vs v2.3 (634us):
- Pass 1 interleaves heads per row-tile; both heads' acc accumulate into
  ONE PSUM row set -> single 8KB AllReduce at pass-1 end (plus a tiny
  warmup AR to absorb core skew before it).
- sv scale path uses [16,CH] contiguous loads + gpsimd partition reduce
  + partition_broadcast (the old svh gather was 4k x 4B descriptor DMAs
  that stalled the acc AllReduce by ~27us).
- k4/k8 int tiles precomputed on vector slack during pass-1; post-topk
  work is just flag broadcast + copy_predicated + one scale mult.
- wo prefetched during pass 1.
- Tail: ctx is AllGathered RAW in bf16 per head (head-0 AG hides under
  pass-2 head-1), cmax AR runs between the AGs, quantization happens
  after the gather on every core, o-proj consumes gathered tiles
  uniformly (no own-tile special case), f1 epilogue runs per chunk.
"""

import numpy as np
import ml_dtypes

import concourse.bass as bass
import concourse.mybir as mybir
import concourse.tile as tile
from concourse import bacc
from concourse import bass_isa
from concourse.bass_utils import run_bass_kernel_spmd

F32 = mybir.dt.float32
BF16 = mybir.dt.bfloat16
I32 = mybir.dt.int32
AX = mybir.AxisListType
OP = mybir.AluOpType
ACTF = mybir.ActivationFunctionType
RED = bass_isa.ReduceOp

NCORES = 8
T = 2048
E = 2048
H = 16
D = 128                   # head dim
HL = H // NCORES          # heads per core = 2
CH = HL * D               # channels per core = 256
NT = T // 128             # 16 row tiles
NE = E // 128             # 16 contraction tiles
Q8 = 127.0
Q4 = 7.0
EPS = 1e-5
NEG = -1e9
RND_C = 12582912.0        # 1.5 * 2**23 round-to-int trick constant
SCALING = float(D) ** -0.5
K_TOP = T // 40           # 51


def _cdiv(a, b):
    return (a + b - 1) // b


def build(mask_mode: str):
    nc = bacc.Bacc("TRN2", target_bir_lowering=False, debug=False,
                   num_devices=NCORES)

    def dt_in(n, s, d):
        return nc.dram_tensor(n, s, d, kind="ExternalInput").ap()

    g = {"mode": mask_mode}
    g["xq_d"] = dt_in("xq", [E, T], BF16)
    g["sx_d"] = dt_in("sx", [T], F32)
    g["sxpp_d"] = dt_in("sxpp", [128, NT], F32)
    for w in ("wq", "wk", "wv", "wo"):
        g[w + "_d"] = dt_in(w, [E, CH], BF16)
    for v in ("swq", "swk", "swo", "qb", "kb", "ob"):
        g[v + "_d"] = dt_in(v, [128, HL], F32)      # pp layout from host
    g["swv_d"] = dt_in("swv", [CH], F32)
    g["vb_d"] = dt_in("vb", [CH], F32)
    if mask_mode == "causal":
        g["tblk_d"] = dt_in("tblk", [128, 128], F32)
        g["tblkT_d"] = dt_in("tblkT", [128, 128], F32)
    if mask_mode == "general":
        g["mask_d"] = dt_in("mask", [T, T], F32)
        g["maskT_d"] = dt_in("maskT", [T, T], F32)
    g["rvr_d"] = dt_in("rvr", [4, 512], F32)

    g["outT_d"] = nc.dram_tensor("outT", [CH, T], F32,
                                 kind="ExternalOutput").ap()

    # internal DRAM (row/token order unless noted)
    g["kmx_in"] = nc.dram_tensor("kmx_in", [2, T], F32).ap()
    g["kmx_out"] = nc.dram_tensor("kmx_out", [2, T], F32,
                                  addr_space="Shared").ap()
    g["ym_in"] = nc.dram_tensor("ym_in", [T], F32).ap()       # pp order!
    g["ym_out"] = nc.dram_tensor("ym_out", [T], F32,
                                 addr_space="Shared").ap()
    g["warm_in"] = nc.dram_tensor("warm_in", [16], F32).ap()
    g["warm_out"] = nc.dram_tensor("warm_out", [16], F32,
                                   addr_space="Shared").ap()
    g["acc_in"] = nc.dram_tensor("acc_in", [T], F32).ap()
    g["acc_out"] = nc.dram_tensor("acc_out", [T], F32,
                                  addr_space="Shared").ap()
    g["ssel_b"] = nc.dram_tensor("ssel_b", [T], BF16).ap()
    g["flg_b"] = nc.dram_tensor("flg_b", [T], I32).ap()
    g["rs4_b"] = nc.dram_tensor("rs4_b", [T], F32).ap()
    g["rs8_b"] = nc.dram_tensor("rs8_b", [T], F32).ap()
    g["svm_b"] = nc.dram_tensor("svm_b", [NT * CH], F32).ap()
    g["sv_b"] = nc.dram_tensor("sv_b", [CH], F32).ap()
    g["z_b"] = nc.dram_tensor("z_b", [2, T], F32).ap()
    g["rz_b"] = nc.dram_tensor("rz_b", [2, T], F32).ap()
    g["cmu_b"] = nc.dram_tensor("cmu_b", [2, T], F32).ap()
    g["rsc_b"] = nc.dram_tensor("rsc_b", [T], F32).ap()
    g["sc_b"] = nc.dram_tensor("sc_b", [T], F32).ap()
    g["cmx_in"] = nc.dram_tensor("cmx_in", [T], F32).ap()
    g["cmx_out"] = nc.dram_tensor("cmx_out", [T], F32,
                                  addr_space="Shared").ap()
    g["ag_in0"] = nc.dram_tensor("ag_in0", [128 * T], BF16).ap()
    g["ag_out0"] = nc.dram_tensor("ag_out0", [NCORES, 128 * T], BF16,
                                  addr_space="Shared").ap()
    g["ag_in1"] = nc.dram_tensor("ag_in1", [128 * T], BF16).ap()
    g["ag_out1"] = nc.dram_tensor("ag_out1", [NCORES, 128 * T], BF16,
                                  addr_space="Shared").ap()
    g["rg"] = [list(range(NCORES))]

    with tile.TileContext(nc) as tc:
        _body(nc, tc, g)
    nc.compile()
    return nc


def _body(nc, tc, g):
    rg = g["rg"]
    causal = g["mode"] == "causal"
    general = g["mode"] == "general"

    def pool(name, bufs=1, space="SBUF"):
        cm = tc.tile_pool(name=name, bufs=bufs, space=space)
        return cm, cm.__enter__()

    # ---------------- persistent pool ----------------
    per_cm, per = pool("per")
    sx_pp = per.tile([128, NT], F32, tag="sxpp")
    nc.sync.dma_start(sx_pp[:], g["sxpp_d"][:])

    def load_pp(dram_pp, tag):
        t_ = per.tile([128, HL], F32, tag=tag)
        nc.sync.dma_start(t_[:], dram_pp[:])
        return t_

    swq_pp = load_pp(g["swq_d"], "swq")
    swk_pp = load_pp(g["swk_d"], "swk")
    swo_pp = load_pp(g["swo_d"], "swo")
    qb_pp = load_pp(g["qb_d"], "qb")
    kb_pp = load_pp(g["kb_d"], "kb")
    ob_pp = load_pp(g["ob_d"], "ob")
    if causal:
        tblk = per.tile([128, 128], F32, tag="tblk")
        tblkT = per.tile([128, 128], F32, tag="tblkT")
        nc.sync.dma_start(tblk[:], g["tblk_d"][:])
        nc.sync.dma_start(tblkT[:], g["tblkT_d"][:])
    ones_sb = per.tile([128, 1], BF16, tag="ones")
    nc.vector.memset(ones_sb[:], 1.0)
    rndc_pp = per.tile([128, 1], F32, tag="rndcpp")
    nc.vector.memset(rndc_pp[:], RND_C)
    swv_rep = per.tile([128, CH], F32, tag="swvrep")
    vb_rep = per.tile([128, CH], F32, tag="vbrep")
    nc.sync.dma_start(swv_rep[:],
                      g["swv_d"].rearrange("(a c) -> a c", a=1)
                      .to_broadcast([128, CH]))
    nc.sync.dma_start(vb_rep[:],
                      g["vb_d"].rearrange("(a c) -> a c", a=1)
                      .to_broadcast([128, CH]))

    qT = [per.tile([128, T], BF16, tag=f"qT{h}", name=f"qT{h}")
          for h in range(HL)]
    k2 = [per.tile([128, T], BF16, tag=f"k2{h}", name=f"k2{h}")
          for h in range(HL)]
    vqi = per.tile([128, NT * CH], BF16, tag="vqi")
    sv_pp = per.tile([128, HL], F32, tag="svpp")
    sy_pp = per.tile([128, NT], F32, tag="sypp")
    rsy_pp = per.tile([128, NT], F32, tag="rsypp")
    kmx4 = per.tile([4, 512], F32, tag="kmx4")
    s8r = per.tile([4, 512], F32, tag="s8r")
    s4r = per.tile([4, 512], F32, tag="s4r")
    rs8r = per.tile([4, 512], F32, tag="rs8r")
    rs4r = per.tile([4, 512], F32, tag="rs4r")
    wo_sb = per.tile([128, NE * CH], BF16, tag="wosb")

    # warm up the exp table set early (one-time ~2.7us load)
    wex = per.tile([1, 8], F32, tag="wex")
    nc.scalar.activation(wex[:], sx_pp[0:1, 0:8], ACTF.Exp)

    # ---------------- wA: yv/v1i (live to gap end) ----------------
    wA_cm, wA = pool("wA")
    yv = wA.tile([128, NT * CH], F32, tag="yv")
    ym_pp = wA.tile([128, NT], F32, tag="ympp")
    kTx = [wA.tile([128, T], F32, tag=f"kTx{h}", name=f"kTx{h}")
           for h in range(HL)]
    kTr = [wA.tile([128, T], BF16, tag=f"kTr{h}", name=f"kTr{h}")
           for h in range(HL)]

    # ---------------- wX: xq + resident weights (die after proj) --------
    wX_cm, wX = pool("wX")
    xq = wX.tile([128, NE * T], BF16, tag="xq")
    wk_sb = wX.tile([128, NE * CH], BF16, tag="wksb")
    wv_sb = wX.tile([128, NE * CH], BF16, tag="wvsb")
    sx_rep = wX.tile([128, T], F32, tag="sxrep")
    # queue plan: sync = xq 0-5,12-15 + sx_rep; gpsimd = xq 6-11 + wk +
    # wv; scalar = streamed wq tiles (inside the Q pass below)
    for et in range(6):
        nc.sync.dma_start(xq[:, et * T:(et + 1) * T],
                          g["xq_d"][et * 128:(et + 1) * 128, :])
    for et in range(6, 12):
        nc.gpsimd.dma_start(xq[:, et * T:(et + 1) * T],
                            g["xq_d"][et * 128:(et + 1) * 128, :])
    for et in range(NE):
        nc.gpsimd.dma_start(wk_sb[:, et * CH:(et + 1) * CH],
                            g["wk_d"][et * 128:(et + 1) * 128, :])
    for et in range(NE):
        nc.gpsimd.dma_start(wv_sb[:, et * CH:(et + 1) * CH],
                            g["wv_d"][et * 128:(et + 1) * 128, :])
    nc.sync.dma_start(sx_rep[:],
                      g["sx_d"].rearrange("(a t) -> a t", a=1)
                      .to_broadcast([128, T]))

    # -------- Q/K projections (half-T PSUM) + V interleaved in K --------
    p1b_cm, p1b = pool("p1b", bufs=3)
    ps1_cm, ps1 = pool("ps1", space="PSUM")
    ps1v_cm, ps1v = pool("ps1v", space="PSUM")

    def v_block(j):
        pV = ps1v.tile([128, CH], F32, tag="pV", name=f"pV{j}")
        for et2 in range(NE):
            nc.tensor.matmul(pV[:],
                             xq[:, et2 * T + j * 128:et2 * T + (j + 1) * 128],
                             wv_sb[:, et2 * CH:(et2 + 1) * CH],
                             start=(et2 == 0), stop=(et2 == NE - 1))
        jsl = slice(j * CH, (j + 1) * CH)
        e3 = p1b.tile([128, CH], F32, tag="e3", bufs=2)
        nc.vector.tensor_scalar(e3[:], pV[:], sx_pp[:, j:j + 1],
                                None, op0=OP.mult)
        nc.vector.tensor_tensor(e3[:], e3[:], swv_rep[:], op=OP.mult)
        nc.vector.tensor_tensor(yv[:, jsl], e3[:], vb_rep[:], op=OP.add)
        nc.vector.tensor_reduce(ym_pp[:, j:j + 1], yv[:, jsl],
                                axis=AX.X, op=OP.max,
                                apply_absolute_value=True)

    ets_v1 = [e for e in range(NE) if e % 4 != 0]   # 12 slots in K half 1
    for proj in ("q", "k"):
        for thalf in range(2):
            tsl = slice(thalf * 1024, (thalf + 1) * 1024)
            pP = [ps1.tile([128, 1024], F32, tag=f"pP{o}",
                           name=f"pP{o}_{proj}{thalf}") for o in range(2)]
            for et in range(NE):
                if proj == "q":
                    we = p1b.tile([128, CH], BF16, tag="wstream")
                    nc.scalar.dma_start(we[:],
                                        g["wq_d"][et * 128:(et + 1) * 128, :])
                    if thalf == 0 and et % 4 == 3:
                        xet = 12 + et // 4
                        nc.scalar.dma_start(
                            xq[:, xet * T:(xet + 1) * T],
                            g["xq_d"][xet * 128:(xet + 1) * 128, :])
                    wsrc = we
                else:
                    wsrc = wk_sb[:, et * CH:(et + 1) * CH]
                for o in range(2):
                    for n2 in range(2):
                        nc.tensor.matmul(
                            pP[o][:, n2 * 512:(n2 + 1) * 512],
                            wsrc[:, o * 128:(o + 1) * 128],
                            xq[:, et * T + thalf * 1024 + n2 * 512:
                               et * T + thalf * 1024 + (n2 + 1) * 512],
                            start=(et == 0), stop=(et == NE - 1))
                # V blocks: 4 late in K half 0, 12 spread over K half 1
                if proj == "k" and thalf == 0 and et >= 12:
                    v_block(et - 12)
                if proj == "k" and thalf == 1 and et in ets_v1:
                    v_block(4 + ets_v1.index(et))
            for o in range(2):
                e1 = p1b.tile([128, 1024], F32, tag="ev1", bufs=2)
                sw = swq_pp if proj == "q" else swk_pp
                bb = qb_pp if proj == "q" else kb_pp
                nc.scalar.activation(e1[:], pP[o][:], ACTF.Copy,
                                     scale=sw[:, o:o + 1])
                nc.vector.tensor_tensor(e1[:], e1[:], sx_rep[:, tsl],
                                        op=OP.mult)
                if proj == "q":
                    nc.vector.tensor_scalar(qT[o][:, tsl], e1[:],
                                            bb[:, o:o + 1], SCALING,
                                            op0=OP.add, op1=OP.mult)
                else:
                    nc.vector.tensor_scalar(kTx[o][:, tsl], e1[:],
                                            bb[:, o:o + 1], None, op0=OP.add)
                    nc.scalar.activation(kTr[o][:, tsl], e1[:],
                                         ACTF.Identity, bias=bb[:, o:o + 1])
    ps1v_cm.__exit__(None, None, None)
    ps1_cm.__exit__(None, None, None)

    # ym AR first (absorbs skew, hides under pass 1); pp order is fine
    # because elementwise max is layout-agnostic if all cores agree
    nc.sync.dma_start(g["ym_in"].rearrange("(p j) -> p j", p=128), ym_pp[:])
    nc.gpsimd.collective_compute("AllReduce", OP.max,
                                 ins=[g["ym_in"][:]], outs=[g["ym_out"][:]],
                                 replica_groups=rg)
    # kmax AR
    kmxs = p1b.tile([128, T], F32, tag="kmxs", bufs=1)
    for hh in range(HL):
        nc.gpsimd.partition_all_reduce(kmxs[:], kTx[hh][:], 128, RED.absmax)
        nc.sync.dma_start(g["kmx_in"][hh, :].rearrange("(a t) -> a t", a=1),
                          kmxs[0:1, :])
    nc.gpsimd.collective_compute("AllReduce", OP.max,
                                 ins=[g["kmx_in"][:]], outs=[g["kmx_out"][:]],
                                 replica_groups=rg)
    p1b_cm.__exit__(None, None, None)
    wX_cm.__exit__(None, None, None)

    # prefetch o-proj weights during pass 1 (sync/scalar queues)
    for et in range(NE):
        (nc.sync if et % 2 == 0 else nc.scalar).dma_start(
            wo_sb[:, et * CH:(et + 1) * CH],
            g["wo_d"][et * 128:(et + 1) * 128, :])

    # wB: k-int tiles + scale replicas (allocated in xq's freed space)
    wB_cm, wB = pool("wB")
    k4i = [wB.tile([128, T], BF16, tag=f"k4i{h}", name=f"k4i{h}")
           for h in range(HL)]
    k8i = [wB.tile([128, T], BF16, tag=f"k8i{h}", name=f"k8i{h}")
           for h in range(HL)]
    rs4_rep = wB.tile([128, T], F32, tag="rs4rep")
    rs8_rep = wB.tile([128, T], F32, tag="rs8rep")

    # ---------------- pass 1 (i outer, heads inner) ----------------
    p5_cm, p5 = pool("p5", bufs=3)
    p5m_cm, p5m = pool("p5m", bufs=2)
    psS_cm, psS_p = pool("psS", bufs=1, space="PSUM")
    psA_cm, psA_p = pool("psA", space="PSUM")
    pA = [psA_p.tile([1, 512], F32, tag=f"pA{n}", name=f"pA{n}")
          for n in range(4)]
    first_wr = [True] * 4
    vmx = p5.tile([128, 8 * CH], F32, tag="vmx", bufs=1)

    def vquant_chunk(i):
        isl = slice(i * CH, (i + 1) * CH)
        d1 = p5.tile([128, CH], F32, tag="d1", bufs=3)
        nc.vector.tensor_scalar(d1[:], yv[:, isl], rsy_pp[:, i:i + 1],
                                RND_C, op0=OP.mult, op1=OP.add)
        nc.vector.tensor_scalar(yv[:, isl], d1[:], RND_C,
                                sy_pp[:, i:i + 1],
                                op0=OP.subtract, op1=OP.mult)

    def kint_quant(h, rrep, dst):
        # dst = round(kTx[h] * rrep) as bf16 ints; TT on idle gpsimd,
        # single fused round on vector
        t8 = p5.tile([128, T], F32, tag="kq8", bufs=2)
        nc.gpsimd.tensor_tensor(t8[:], kTx[h][:], rrep[:], op=OP.mult)
        nc.vector.tensor_scalar(dst[:], t8[:], RND_C, RND_C,
                                op0=OP.add, op1=OP.subtract)

    for i in range(NT):
        c_cols = (i + 1) * 128 if causal else T
        nch = _cdiv(c_cols, 512)
        diag_n, diag_off = (i * 128) // 512, (i * 128) % 512
        if general:
            mrow = p5m.tile([128, T], F32, tag="mrow")
            nc.sync.dma_start(mrow[:],
                              g["mask_d"][i * 128:(i + 1) * 128, :])
        for h in range(HL):
            zz = p5.tile([128, 4], F32, tag=f"zz{h}")
            pp = []
            for n in range(nch):
                w = min(512, c_cols - n * 512)
                psS = psS_p.tile([128, 512], F32, tag=f"pS{h}", bufs=2,
                                 name=f"pS_{h}_{i}_{n}")
                nc.tensor.matmul(psS[:, :w],
                                 qT[h][:, i * 128:(i + 1) * 128],
                                 kTr[h][:, n * 512:n * 512 + w],
                                 start=True, stop=True)
                if causal and n == diag_n:
                    nc.vector.tensor_tensor(psS[:, diag_off:diag_off + 128],
                                            psS[:, diag_off:diag_off + 128],
                                            tblk[:], op=OP.add)
                elif general:
                    nc.vector.tensor_tensor(psS[:, :w], psS[:, :w],
                                            mrow[:, n * 512:n * 512 + w],
                                            op=OP.add)
                p1t = p5.tile([128, 512], BF16, tag=f"p1t{h}", bufs=4,
                              name=f"p1t_{h}_{i}_{n}")
                nc.scalar.activation(p1t[:, :w], psS[:, :w], ACTF.Exp,
                                     bias=0.0, scale=1.0,
                                     accum_out=zz[:, n:n + 1])
                if w < 512:
                    nc.vector.memset(p1t[:, w:], 0.0)
                pp.append(p1t)
            z = p5.tile([128, 1], F32, tag=f"z{h}")
            if nch == 1:
                nc.vector.tensor_copy(z[:], zz[:, 0:1])
            else:
                nc.vector.tensor_reduce(z[:], zz[:, :nch], axis=AX.X,
                                        op=OP.add)
            rz = p5.tile([128, 1], BF16, tag=f"rz{h}")
            with nc.allow_low_precision(reason="bf16 matmul feed"):
                nc.vector.reciprocal(rz[:], z[:])
            for n in range(nch):
                nc.tensor.matmul(pA[n][:], rz[:], pp[n][:],
                                 start=first_wr[n],
                                 stop=(i == NT - 1 and h == HL - 1))
                first_wr[n] = False

        # ---- i-driven slack work ----
        if i == 7:
            # quant-scale prep (ym/kmx ARs done by now)
            ymf = p5.tile([128, NT], F32, tag="ymf", bufs=1)
            nc.sync.dma_start(
                ymf[:], g["ym_out"].rearrange("(p j) -> p j", p=128))
            nc.vector.tensor_scalar(sy_pp[:], ymf[:], EPS, 1.0 / Q8,
                                    op0=OP.max, op1=OP.mult)
            nc.vector.reciprocal(rsy_pp[:], sy_pp[:])
        if i == 10:
            # k-quant scales (kmx AR done by now)
            nc.sync.dma_start(
                kmx4[:],
                g["kmx_out"][0, :].rearrange("(r s) -> r s", r=4))
            km2 = p5.tile([4, 512], F32, tag="km2", bufs=1)
            nc.sync.dma_start(
                km2[:],
                g["kmx_out"][1, :].rearrange("(r s) -> r s", r=4))
            nc.vector.tensor_tensor(kmx4[:], kmx4[:], km2[:],
                                    op=OP.max)
            nc.vector.tensor_scalar(s8r[:], kmx4[:], 1.0 / Q8, EPS,
                                    op0=OP.mult, op1=OP.max)
            nc.vector.tensor_scalar(s4r[:], kmx4[:], EPS, 1.0 / Q4,
                                    op0=OP.max, op1=OP.mult)
            nc.vector.reciprocal(rs8r[:], s8r[:])
            nc.vector.reciprocal(rs4r[:], s4r[:])
            # broadcast reciprocal-scale rows for the kint precompute
            nc.scalar.dma_start(g["rs4_b"].rearrange("(r s) -> r s", r=4),
                                rs4r[:])
            nc.scalar.dma_start(g["rs8_b"].rearrange("(r s) -> r s", r=4),
                                rs8r[:])
            nc.scalar.dma_start(rs4_rep[:],
                                g["rs4_b"].rearrange("(a t) -> a t", a=1)
                                .to_broadcast([128, T]))
            nc.scalar.dma_start(rs8_rep[:],
                                g["rs8_b"].rearrange("(a t) -> a t", a=1)
                                .to_broadcast([128, T]))
        if 8 <= i <= 15:
            vquant_chunk(2 * (i - 8))
            vquant_chunk(2 * (i - 8) + 1)
        if i == 11:
            kint_quant(0, rs4_rep, k4i[0])
        if i == 12:
            kint_quant(0, rs8_rep, k8i[0])
        if i == 13:
            kint_quant(1, rs4_rep, k4i[1])
        if i == 14:
            kint_quant(1, rs8_rep, k8i[1])
        if i == 12:
            nc.gpsimd.partition_all_reduce(vmx[:], yv[:, :8 * CH],
                                           128, RED.absmax)
            nc.sync.dma_start(
                g["svm_b"][0:8 * CH].rearrange("(a c) -> a c", a=1),
                vmx[0:1, :])
        if i == 14:
            # warmup AR to absorb inter-core skew before the acc AR
            wtl = p5.tile([1, 16], F32, tag="wtl", bufs=1)
            nc.vector.memset(wtl[:], 1.0)
            nc.sync.dma_start(
                g["warm_in"].rearrange("(a t) -> a t", a=1), wtl[:])
            nc.gpsimd.collective_compute(
                "AllReduce", OP.max,
                ins=[g["warm_in"][:]], outs=[g["warm_out"][:]],
                replica_groups=rg)

    # ---- end i loop: single acc AR for both heads ----
    accs = p5.tile([1, T], F32, tag="accs", bufs=1)
    for n in range(4):
        nc.vector.tensor_copy(accs[:, n * 512:(n + 1) * 512], pA[n][:])
    nc.sync.dma_start(g["acc_in"].rearrange("(a t) -> a t", a=1), accs[:])
    nc.gpsimd.collective_compute("AllReduce", OP.add,
                                 ins=[g["acc_in"][:]], outs=[g["acc_out"][:]],
                                 replica_groups=rg)
    nc.gpsimd.partition_all_reduce(vmx[:], yv[:, 8 * CH:],
                                   128, RED.absmax)
    nc.sync.dma_start(
        g["svm_b"][8 * CH:].rearrange("(a c) -> a c", a=1),
        vmx[0:1, :])
    psA_cm.__exit__(None, None, None)
    psS_cm.__exit__(None, None, None)
    p5m_cm.__exit__(None, None, None)
    p5_cm.__exit__(None, None, None)

    # -------- sv scales + vqi (hide under acc AR) + topk + k2 --------
    p6_cm, p6 = pool("p6")
    sv16 = p6.tile([16, CH], F32, tag="sv16")
    nc.sync.dma_start(sv16[:],
                      g["svm_b"].rearrange("(j c) -> j c", j=16))
    sv16r = p6.tile([16, CH], F32, tag="sv16r")
    nc.gpsimd.partition_all_reduce(sv16r[:], sv16[:], 16, RED.max)
    sv_row = p6.tile([1, CH], F32, tag="svrow")
    nc.vector.tensor_scalar(sv_row[:], sv16r[0:1, :], EPS, 1.0 / Q8,
                            op0=OP.max, op1=OP.mult)
    rsv_row = p6.tile([1, CH], F32, tag="rsvrow")
    nc.vector.reciprocal(rsv_row[:], sv_row[:])
    rsv_rep = p6.tile([128, CH], F32, tag="rsvrep")
    nc.gpsimd.partition_broadcast(rsv_rep[:], rsv_row[:])
    nc.sync.dma_start(g["sv_b"].rearrange("(a c) -> a c", a=1), sv_row[:])
    nc.sync.dma_start(sv_pp[:],
                      g["sv_b"].rearrange("(h p) -> p h", p=128))
    # vqi quant: TT on gpsimd, fused round on vector (hides under acc AR)
    for j in range(NT):
        jsl = slice(j * CH, (j + 1) * CH)
        m1 = p6.tile([128, CH], F32, tag="m1", bufs=3)
        nc.gpsimd.tensor_tensor(m1[:], yv[:, jsl], rsv_rep[:], op=OP.mult)
        nc.vector.tensor_scalar(vqi[:, jsl], m1[:], RND_C, RND_C,
                                op0=OP.add, op1=OP.subtract)

    acc4 = p6.tile([4, 512], F32, tag="acc4")
    nc.sync.dma_start(acc4[:], g["acc_out"].rearrange("(r s) -> r s", r=4))
    rvr = p6.tile([4, 512], F32, tag="rvr")
    nc.sync.dma_start(rvr[:], g["rvr_d"][:])
    nc.vector.tensor_tensor(acc4[:], acc4[:], rvr[:], op=OP.mult)
    tkw = p6.tile([4, 512], F32, tag="tkw")
    ton = acc4[:]
    for k_on in range(0, K_TOP, 8):
        k_this = min(k_on + 8, K_TOP) - k_on
        mx8 = p6.tile([4, 8], F32, tag="mx8")
        nc.vector.max(out=mx8[:], in_=ton)
        if k_this < 8:
            nc.vector.memset(mx8[:, k_this:], 0)
        nc.vector.match_replace(out=tkw[:], in_to_replace=mx8[:],
                                in_values=ton, imm_value=0)
        ton = tkw[:]
    nc.vector.tensor_sub(out=tkw[:], in0=acc4[:], in1=tkw[:])
    flg4 = p6.tile([4, 512], I32, tag="flg4")
    nc.vector.tensor_scalar(flg4[:], tkw[:], 0.0, None, op0=OP.is_gt)
    nc.scalar.dma_start(g["flg_b"].rearrange("(r s) -> r s", r=4), flg4[:])
    nc.vector.copy_predicated(s4r[:], flg4[:], s8r[:])
    ssel4 = p6.tile([4, 512], BF16, tag="ssel4")
    nc.vector.tensor_copy(ssel4[:], s4r[:])
    nc.sync.dma_start(g["ssel_b"].rearrange("(r s) -> r s", r=4), ssel4[:])
    ssel_rep = p6.tile([128, T], BF16, tag="sselrep")
    flg_rep = p6.tile([128, T], I32, tag="flgrep")
    # chunked broadcasts + chunked k2 build so pass 2 starts on chunk 0
    for cn in range(4):
        csl = slice(cn * 512, (cn + 1) * 512)
        nc.scalar.dma_start(flg_rep[:, csl],
                            g["flg_b"][csl].rearrange("(a t) -> a t", a=1)
                            .to_broadcast([128, 512]))
        nc.sync.dma_start(ssel_rep[:, csl],
                          g["ssel_b"][csl].rearrange("(a t) -> a t", a=1)
                          .to_broadcast([128, 512]))
    for hh in range(HL):
        for cn in range(4):
            csl = slice(cn * 512, (cn + 1) * 512)
            nc.vector.copy_predicated(k4i[hh][:, csl], flg_rep[:, csl],
                                      k8i[hh][:, csl])
            nc.vector.tensor_tensor(k2[hh][:, csl], k4i[hh][:, csl],
                                    ssel_rep[:, csl], op=OP.mult)
    p6_cm.__exit__(None, None, None)
    wB_cm.__exit__(None, None, None)
    wA_cm.__exit__(None, None, None)

    # ---------------- pass 2 (flipped, j-outer half rows) ----------------
    p7_cm, p7 = pool("p7", bufs=3)
    p7m_cm, p7m = pool("p7m", bufs=2)
    ps2S_cm, ps2S = pool("ps2s", bufs=2, space="PSUM")
    ps2C_cm, ps2C = pool("ps2c", space="PSUM")
    ps2Z_cm, ps2Z = pool("ps2z", space="PSUM")
    ctxU = [p7.tile([128, T], F32, tag=f"ctxU{h}", name=f"ctxU{h}", bufs=1)
            for h in range(HL)]
    rz_rep = [p7.tile([128, T], F32, tag="rzrep", name=f"rzrep{h}", bufs=1)
              for h in range(HL)]
    ctxb = [p7.tile([128, T], BF16, tag=f"ctxb{h}", name=f"ctxb{h}", bufs=1)
            for h in range(HL)]
    cmup = [p7.tile([128, T], F32, tag=f"cmup{h}", name=f"cmup{h}", bufs=1)
            for h in range(HL)]
    cm4 = [p7.tile([4, 512], F32, tag=f"cm4{h}", name=f"cm4{h}", bufs=1)
           for h in range(HL)]

    for h in range(HL):
        for half in range(2):
            tlo = half * 1024
            jmax = (8 + 8 * half) if causal else NT
            psC = [ps2C.tile([128, 512], F32, tag=f"pC{s}",
                             name=f"pC{s}_{h}_{half}") for s in range(2)]
            psZ = [ps2Z.tile([1, 512], F32, tag=f"pZ{s}",
                             name=f"pZ{s}_{h}_{half}") for s in range(2)]
            for j in range(jmax):
                off = max(0, j * 128 - tlo) if causal else 0
                psSX = ps2S.tile([128, 1024], F32, tag="pSX",
                                 name=f"pSX{h}_{half}_{j}")
                for s2 in range(2):
                    lo2 = max(off, s2 * 512)
                    if lo2 >= (s2 + 1) * 512:
                        continue
                    nc.tensor.matmul(psSX[:, lo2:(s2 + 1) * 512],
                                     k2[h][:, j * 128:(j + 1) * 128],
                                     qT[h][:, tlo + lo2:
                                           tlo + (s2 + 1) * 512],
                                     start=True, stop=True)
                if causal and j * 128 >= tlo:
                    nc.vector.tensor_tensor(psSX[:, off:off + 128],
                                            psSX[:, off:off + 128],
                                            tblkT[:], op=OP.add)
                elif general:
                    mrowT = p7m.tile([128, 1024], F32, tag="mrowT")
                    nc.sync.dma_start(
                        mrowT[:], g["maskT_d"][j * 128:(j + 1) * 128,
                                               tlo:tlo + 1024])
                    nc.vector.tensor_tensor(psSX[:], psSX[:], mrowT[:],
                                            op=OP.add)
                p2t = p7.tile([128, 1024], BF16, tag="p2t", bufs=3,
                              name=f"p2t{h}_{half}_{j}")
                if off > 0:
                    nc.vector.memset(p2t[:, :off], 0.0)
                nc.scalar.activation(p2t[:, off:], psSX[:, off:], ACTF.Exp,
                                     bias=0.0, scale=1.0)
                for s in range(2):
                    n = 2 * half + s
                    jm = min(4 * n + 4, jmax) if causal else jmax
                    if j >= jm:
                        continue
                    ssl = slice(s * 512, (s + 1) * 512)
                    nc.tensor.matmul(psZ[s][:], ones_sb[:], p2t[:, ssl],
                                     start=(j == 0), stop=(j == jm - 1))
                    nc.tensor.matmul(psC[s][:],
                                     vqi[:, j * CH + h * 128:
                                         j * CH + (h + 1) * 128],
                                     p2t[:, ssl],
                                     start=(j == 0), stop=(j == jm - 1))
            for s in range(2):
                nsl = slice(tlo + s * 512, tlo + (s + 1) * 512)
                nc.vector.tensor_scalar(ctxU[h][:, nsl], psC[s][:],
                                        sv_pp[:, h:h + 1], None, op0=OP.mult)
                zst = p7.tile([1, 512], F32, tag="zst", bufs=2,
                              name=f"zst{h}_{half}_{s}")
                nc.vector.tensor_copy(zst[:], psZ[s][:])
                nc.sync.dma_start(
                    g["z_b"][h, tlo + s * 512:tlo + (s + 1) * 512]
                    .rearrange("(a t) -> a t", a=1), zst[:])
            # per-half unnormalized channel-absmax (hides under next half)
            nc.gpsimd.partition_all_reduce(cmup[h][:, tlo:tlo + 1024],
                                           ctxU[h][:, tlo:tlo + 1024],
                                           128, RED.absmax)
        nc.sync.dma_start(g["cmu_b"][h, :].rearrange("(a t) -> a t", a=1),
                          cmup[h][0:1, :])
        # z -> [4,512] recip -> rz row -> broadcast prefetch
        z4 = p7.tile([4, 512], F32, tag="z4", name=f"z4_{h}", bufs=2)
        nc.sync.dma_start(z4[:],
                          g["z_b"][h, :].rearrange("(r s) -> r s", r=4))
        rz4 = p7.tile([4, 512], F32, tag="rz4", name=f"rz4_{h}", bufs=2)
        nc.vector.reciprocal(rz4[:], z4[:])
        # cmax row for this head first: the cmx AR input must be ready
        # BEFORE ag_in1 so the scheduler runs the AR ahead of AG1
        nc.sync.dma_start(cm4[h][:],
                          g["cmu_b"][h, :].rearrange("(r s) -> r s", r=4))
        nc.vector.tensor_tensor(cm4[h][:], cm4[h][:], rz4[:], op=OP.mult)
        if h == 1:
            cmr = p7.tile([4, 512], F32, tag="cmr", bufs=1)
            nc.vector.tensor_tensor(cmr[:], cm4[0][:], cm4[1][:],
                                    op=OP.max)
            nc.sync.dma_start(g["cmx_in"].rearrange("(r s) -> r s", r=4),
                              cmr[:])
        nc.sync.dma_start(g["rz_b"][h, :].rearrange("(r s) -> r s", r=4),
                          rz4[:])
        nc.sync.dma_start(
            rz_rep[h][:], g["rz_b"][h, :].rearrange("(a t) -> a t", a=1)
            .to_broadcast([128, T]))
        nc.vector.tensor_tensor(ctxU[h][:], ctxU[h][:], rz_rep[h][:],
                                op=OP.mult)
        # raw bf16 ctx for the AllGather
        nc.vector.tensor_copy(ctxb[h][:], ctxU[h][:])
        ag_in = g["ag_in0"] if h == 0 else g["ag_in1"]
        nc.sync.dma_start(ag_in.rearrange("(p t) -> p t", p=128),
                          ctxb[h][:])
        if h == 0:
            nc.gpsimd.collective_compute(
                "AllGather", OP.bypass,
                ins=[g["ag_in0"][:]], outs=[g["ag_out0"][:]],
                replica_groups=rg)
    # cmax AR (before AG1 so o-proj scales arrive ASAP)
    nc.gpsimd.collective_compute("AllReduce", OP.max,
                                 ins=[g["cmx_in"][:]], outs=[g["cmx_out"][:]],
                                 replica_groups=rg)
    nc.gpsimd.collective_compute("AllGather", OP.bypass,
                                 ins=[g["ag_in1"][:]], outs=[g["ag_out1"][:]],
                                 replica_groups=rg)
    ps2Z_cm.__exit__(None, None, None)
    ps2C_cm.__exit__(None, None, None)
    ps2S_cm.__exit__(None, None, None)
    p7m_cm.__exit__(None, None, None)
    p7_cm.__exit__(None, None, None)

    # -------- gathered-tile loads + quant scales --------
    ps9_cm, ps9 = pool("ps9", space="PSUM")
    p9_cm, p9 = pool("p9", bufs=2)
    ag30 = g["ag_out0"].rearrange("c (p t) -> c p t", p=128)
    ag31 = g["ag_out1"].rearrange("c (p t) -> c p t", p=128)
    # head-0 tiles: resident, loaded on scalar as soon as AG0 lands
    # (hides under the cmax AR); head-1 tiles stream via gpsimd queue
    ct0 = [p9.tile([128, T], BF16, tag=f"ct0_{c2}", bufs=1,
                   name=f"ct0_{c2}") for c2 in range(NCORES)]
    for c2 in range(NCORES):
        nc.scalar.dma_start(ct0[c2][:], ag30[c2])

    cmx4 = p9.tile([4, 512], F32, tag="cmx4", bufs=1)
    nc.sync.dma_start(cmx4[:],
                      g["cmx_out"].rearrange("(r s) -> r s", r=4))
    sc4 = p9.tile([4, 512], F32, tag="sc4", bufs=1)
    nc.vector.tensor_scalar(sc4[:], cmx4[:], EPS, 1.0 / Q8,
                            op0=OP.max, op1=OP.mult)
    nc.sync.dma_start(g["sc_b"].rearrange("(r s) -> r s", r=4), sc4[:])
    rsc4 = p9.tile([4, 512], F32, tag="rsc4", bufs=1)
    nc.vector.reciprocal(rsc4[:], sc4[:])
    nc.scalar.dma_start(g["rsc_b"].rearrange("(r s) -> r s", r=4), rsc4[:])
    rsc_rep = p9.tile([128, T], F32, tag="rscrep", bufs=1)
    nc.scalar.dma_start(rsc_rep[:],
                        g["rsc_b"].rearrange("(a t) -> a t", a=1)
                        .to_broadcast([128, T]))
    sc_rep = p9.tile([128, T], F32, tag="screp", bufs=1)
    nc.sync.dma_start(sc_rep[:],
                      g["sc_b"].rearrange("(a t) -> a t", a=1)
                      .to_broadcast([128, T]))

    # -------- output projection over gathered tiles --------
    pO = [ps9.tile([128, T], F32, tag=f"pO{o}", name=f"pO{o}")
          for o in range(2)]
    order = [(c2, hh) for hh in range(HL) for c2 in range(NCORES)]
    nord = len(order)
    for idx, (c2, hh) in enumerate(order):
        if hh == 0:
            ct = ct0[c2]
        else:
            ct = p9.tile([128, T], BF16, tag="ct1", bufs=4,
                         name=f"ct1_{c2}")
            nc.gpsimd.dma_start(ct[:], ag31[c2])
        ctf = p9.tile([128, T], F32, tag="ctf", bufs=3,
                      name=f"ctf{hh}_{c2}")
        (nc.vector if idx % 2 == 0 else nc.gpsimd).tensor_tensor(
            ctf[:], ct[:], rsc_rep[:], op=OP.mult)
        cti = p9.tile([128, T], BF16, tag="cti", bufs=4,
                      name=f"cti{hh}_{c2}")
        nc.vector.tensor_scalar(cti[:], ctf[:], RND_C, RND_C,
                                op0=OP.add, op1=OP.subtract)
        gidx = 2 * c2 + hh
        last = idx == nord - 1
        for o in range(2):
            wsl = slice(gidx * CH + o * 128, gidx * CH + (o + 1) * 128)
            for nn in range(4):
                nsl = slice(nn * 512, (nn + 1) * 512)
                nc.tensor.matmul(pO[o][:, nsl], wo_sb[:, wsl], cti[:, nsl],
                                 start=(idx == 0), stop=last)
                if last:
                    # per-chunk epilogue overlaps the remaining matmuls
                    f1 = p9.tile([128, 512], F32, tag="f1", bufs=4,
                                 name=f"f1{o}_{nn}")
                    nc.vector.tensor_tensor(f1[:], pO[o][:, nsl],
                                            sc_rep[:, nsl], op=OP.mult)
                    nc.vector.tensor_scalar(f1[:], f1[:],
                                            swo_pp[:, o:o + 1],
                                            ob_pp[:, o:o + 1],
                                            op0=OP.mult, op1=OP.add)
                    nc.sync.dma_start(g["outT_d"][o * 128:(o + 1) * 128,
                                                  nsl], f1[:])
    p9_cm.__exit__(None, None, None)
    ps9_cm.__exit__(None, None, None)
    per_cm.__exit__(None, None, None)


# ==================== host side ====================

_CACHE = {}


def _get_nc(mode):
    if mode not in _CACHE:
        _CACHE[mode] = build(mode)
    return _CACHE[mode]


def _quant_w(w):
    amax = np.max(np.abs(w), axis=-1, keepdims=True)
    s = np.maximum(amax, np.float32(EPS)) / np.float32(Q8)
    wi = np.round((w / s).astype(np.float32))
    return wi, s[:, 0].astype(np.float32)


def _pp(vec, cols):
    # [cols*128] token/channel-order vector -> [128, cols] per-partition
    return np.ascontiguousarray(np.asarray(vec, np.float32)
                                .reshape(cols, 128).T)


def kernel(hidden_states, attention_mask, q_w, q_b, k_w, k_b, v_w, v_b,
           o_w, o_b, num_heads):
    hidden_states = np.asarray(hidden_states, dtype=np.float32)
    attention_mask = np.asarray(attention_mask, dtype=np.float32)
    assert int(num_heads) == H
    B, T_, E_ = hidden_states.shape
    assert (B, T_, E_) == (1, T, E)

    x = np.ascontiguousarray(hidden_states[0])        # [T, E]
    sx = (np.maximum(np.abs(x).max(axis=-1, keepdims=True),
                     np.float32(EPS)) / np.float32(Q8)).astype(np.float32)
    xi = np.round(x / sx)                             # ints, bf16-exact
    xqT = np.ascontiguousarray(xi.T).astype(ml_dtypes.bfloat16)   # [E, T]

    causal_ref = np.triu(np.full((T, T), np.float32(NEG), np.float32), k=1)
    mfull = np.ascontiguousarray(attention_mask[0, 0])
    if np.array_equal(mfull, causal_ref):
        mode = "causal"
    elif not mfull.any():
        mode = "zero"
    else:
        mode = "general"

    nc = _get_nc(mode)

    wqi, sq = _quant_w(np.asarray(q_w, np.float32))
    wki, sk = _quant_w(np.asarray(k_w, np.float32))
    wvi, sv = _quant_w(np.asarray(v_w, np.float32))
    woi, so = _quant_w(np.asarray(o_w, np.float32))

    tblk = np.triu(np.full((128, 128), np.float32(NEG), np.float32), k=1)
    tblkT = np.ascontiguousarray(tblk.T)
    rowvec = np.float32(T) - np.arange(T, dtype=np.float32)
    rvr = (np.float32(1.0) / rowvec).reshape(4, 512).astype(np.float32)

    in_maps = []
    for c in range(NCORES):
        ch = slice(c * CH, (c + 1) * CH)
        woT = np.ascontiguousarray(woi[ch, :].T)      # [E, CH]
        im = dict(
            xq=xqT,
            sx=np.ascontiguousarray(sx[:, 0]),
            sxpp=_pp(sx[:, 0], NT),
            wq=np.ascontiguousarray(wqi[ch, :].T).astype(ml_dtypes.bfloat16),
            wk=np.ascontiguousarray(wki[ch, :].T).astype(ml_dtypes.bfloat16),
            wv=np.ascontiguousarray(wvi[ch, :].T).astype(ml_dtypes.bfloat16),
            wo=woT.astype(ml_dtypes.bfloat16),
            swq=_pp(sq[ch], HL),
            swk=_pp(sk[ch], HL),
            swo=_pp(so[ch], HL),
            swv=np.ascontiguousarray(sv[ch]),
            qb=_pp(np.asarray(q_b, np.float32)[ch], HL),
            kb=_pp(np.asarray(k_b, np.float32)[ch], HL),
            ob=_pp(np.asarray(o_b, np.float32)[ch], HL),
            vb=np.ascontiguousarray(np.asarray(v_b, np.float32)[ch]),
            rvr=rvr,
        )
        if mode == "causal":
            im["tblk"] = tblk
            im["tblkT"] = tblkT
        if mode == "general":
            im["mask"] = mfull
            im["maskT"] = np.ascontiguousarray(mfull.T)
        in_maps.append(im)

    res = run_bass_kernel_spmd(nc, in_maps, list(range(NCORES)))
    kernel.last_results = res.results
    out = np.empty((T, E), dtype=np.float32)
    for c in range(NCORES):
        out[:, c * CH:(c + 1) * CH] = res.results[c]["outT"].T
    return out.reshape(1, T, E)
